# revision 1
# baseline (speedup 1.0000x reference)
"""Contrastive loss (CLIP-style BCE) on 8 Trainium2 NeuronCores.

Strategy: data-parallel over the batch dim. Each core takes a 128-row shard
of img_features (75.5 MB) with batch rows mapped to the 128 SBUF
partitions, so every pooling DMA moves a [128, ncb*576] tile whose
per-partition source is one contiguous ncb*2304-byte run — large
descriptors, single HWDGE ring, ~full streaming rate. The [128, 1024]
logits slice, BCE partials, and the 128-way reduction all run per core;
the host sums the 8 partial scalars and divides by B*B.

Keys to keeping the stream saturated (found via TimelineSim traces + HW
rep-differenced timing):
- One activation-table set for the whole kernel: bacc's table-load pass
  maps each function to the FIRST act_info.json set containing it, which
  ping-pongs natural_log <-> exp_and_others around every rsqrt (1.28 us
  per reload, 22 reloads originally). We patch bacc's view of the tables
  to subtract natural_log_exp_and_others' functions from sets listed
  before it; every function then first-matches that one set. Set ids and
  real table contents are unchanged.
- Ln/Exp clustered: one batched rsqrt for all 8 text tiles, one tiny
  cluster in the tail.
- Per-channel [128, 576] reductions split DVE/ACT 50:50, with
  text norms, targets (is_equal chain), and PSUM->SBUF copies on DVE so
  ACT keeps pace.
- Text/labels prefetched ahead of the pooling stream, all on the sync
  HWDGE ring (leaving the ACT-issued ring idle measured -6us vs dual-ring); pooling tile sizes taper ([8]*28, 8,8,4,4,4,4) so the final
  reduce backlog drains quickly after the last DMA.
- The pooled-row rsqrt stays off the critical path: transposes feed RAW
  pooled sums to the gram matmuls and the row norm is applied via Exp's
  per-partition scale operand plus one [128,1] fixup on the x*z term.

Runtime notes (bisected on this axon/fakenrt stack):
- PE is_transpose matmuls and InstTensorTensorReduce crash the exec unit;
  transposes are done as regular matmuls against identity.
- float32r matmul operands and SWDGE accumulate/cast DMAs all measured
  SLOWER on HW than this plain-f32 HWDGE pipeline, despite the cost model
  predicting otherwise.
- Softplus/Sqrt have no single-table path here; softplus = ln(exp(x)+1)
  (|x| <= 1/0.07 so exp is safe), rsqrt = exp(-0.5*ln(x)) + one Newton
  step.
"""

import numpy as np

import concourse.bacc as bacc
import concourse.hw_specs as hw_specs
import concourse.mybir as mybir
import concourse.tile as tile
from concourse.bass_utils import run_bass_kernel_spmd
from concourse.masks import make_identity

_PREF_SET = "natural_log_exp_and_others"


def _pinned_tables(arch):
    tabs = hw_specs.get_activation_tables(arch)
    try:
        pref = tabs.get(_PREF_SET)
        if pref is None:
            return tabs
        out = {}
        seen_pref = False
        for name, funcs in tabs.items():
            if name == _PREF_SET:
                out[name] = funcs
                seen_pref = True
            else:
                out[name] = funcs if seen_pref else (funcs - pref)
        return out
    except Exception:
        return tabs


bacc.get_activation_tables = _pinned_tables

N_CORES = 8
B, C, H, W = 1024, 256, 24, 24
HW = H * W  # 576
BS = B // N_CORES  # 128 rows per core
P = 128
TEMP = 0.07
INV_TEMP = 1.0 / TEMP
NT = B // P  # 8 text tiles
ACT_SET = frozenset({1, 3, 5, 7, 9, 11, 13, 15})  # c % 16 -> ACT reduce (8/16)

F32 = mybir.dt.float32
ALU = mybir.AluOpType
ACT = mybir.ActivationFunctionType
AX = mybir.AxisListType

_NC_CACHE = []


def _emit_newton(nc, small, y0, ns, out_rv, tag, width):
    """out_rv = y0 * (1.5 - 0.5 * ns * y0^2) — Newton step for rsqrt."""
    t1 = small.tile([P, width], F32, tag=f"{tag}_t1", name=f"{tag}_t1")
    nc.vector.tensor_mul(t1, y0, y0)
    nc.vector.tensor_mul(t1, t1, ns)
    nc.vector.tensor_scalar(
        out=t1, in0=t1, scalar1=-0.5, scalar2=1.5, op0=ALU.mult, op1=ALU.add
    )
    nc.vector.tensor_mul(out_rv, y0, t1)


def _emit_body(nc, pools, identity, ones, img, txt, lab_row, lab_all, out, cfg):
    consts, big, ascrp, txtp, small, persist, psum_tp, psum_g = pools
    act_set = cfg.get("act_set", ACT_SET)
    sizes = cfg.get("sizes")
    if sizes is None:
        sizes = [8] * 28 + [8, 8, 4, 4, 4, 4]
    assert sum(sizes) == C

    # ---- prefetch: text tiles + labels ahead of the pooling stream ----
    txt_sb = [
        txtp.tile([P, C], F32, tag=f"ttin{tb}", name=f"ttin{tb}") for tb in range(NT)
    ]
    for tb in range(NT):
        nc.sync.dma_start(out=txt_sb[tb], in_=txt[tb * P : (tb + 1) * P, :])
    lab_row_sb = small.tile([P, 1], F32, tag="lab_row_sb", name="lab_row_sb")
    nc.sync.dma_start(out=lab_row_sb, in_=lab_row)
    tgt = persist.tile([P, B], F32, tag="tgt", name="tgt")
    nc.sync.dma_start(out=tgt, in_=lab_all.to_broadcast([P, B]))

    # ---- pooling stream on sync ring ----
    pooled = persist.tile([P, C], F32, tag="pooled", name="pooled")
    c = 0
    for t, sz in enumerate(sizes):
        it = big.tile([P, sz, HW], F32, tag="imgin", name="imgin")
        # max_dma_last_dim=2304 splits each partition line into two 9216-B
        # descriptors; the finer SDMA interleave measured -9.4 us/rep
        nc.sync.dma_start(out=it, in_=img[:, c : c + sz, :], max_dma_last_dim=2304)
        for j in range(sz):
            chunk = it[:, j, :]
            if (c % 16) in act_set:
                ascr = ascrp.tile([P, HW], F32, tag="ascr", name="ascr")
                nc.scalar.activation(
                    ascr, chunk, ACT.Identity, accum_out=pooled[:, c : c + 1]
                )
            else:
                nc.vector.reduce_sum(out=pooled[:, c : c + 1], in_=chunk, axis=AX.X)
            c += 1

    # ---- targets: tgt = (lab_all == lab_row) * (1/T), one DVE op ----
    nc.vector.tensor_scalar(
        out=tgt,
        in0=tgt,
        scalar1=lab_row_sb,
        scalar2=INV_TEMP,
        op0=ALU.is_equal,
        op1=ALU.mult,
    )

    # ---- text norms (DVE) + one batched rsqrt ----
    tns = small.tile([P, NT], F32, tag="tns", name="tns")
    for tb in range(NT):
        tsq = txtp.tile([P, C], F32, tag="tsq", name="tsq")
        nc.vector.tensor_mul(tsq, txt_sb[tb], txt_sb[tb])
        nc.vector.reduce_sum(out=tns[:, tb : tb + 1], in_=tsq, axis=AX.X)
    ty0 = small.tile([P, NT], F32, tag="ty0", name="ty0")
    nc.scalar.activation(ty0, tns, ACT.Ln)
    nc.scalar.activation(ty0, ty0, ACT.Exp, scale=-0.5)
    trv = small.tile([P, NT], F32, tag="trv", name="trv")
    _emit_newton(nc, small, ty0, tns, trv, "trsq", NT)

    # normalize in place, then transpose to [C, B] layout via PE
    txtT = [
        persist.tile([P, B], F32, tag=f"txtT{cb}", name=f"txtT{cb}") for cb in range(2)
    ]
    for tb in range(NT):
        nc.vector.tensor_scalar_mul(txt_sb[tb], txt_sb[tb], trv[:, tb : tb + 1])
        for cb in range(2):
            pt = psum_tp.tile([P, P], F32, tag="pt", name="pt")
            nc.tensor.matmul(
                pt,
                txt_sb[tb][:, cb * P : (cb + 1) * P],
                identity,
                start=True,
                stop=True,
            )
            nc.vector.tensor_copy(txtT[cb][:, tb * P : (tb + 1) * P], pt)

    # ---- pooled row norms; rsqrt runs beside the transposes/matmuls ----
    ns = small.tile([P, 1], F32, tag="ns", name="ns")
    psq_scr = small.tile([P, C], F32, tag="psq_scr", name="psq_scr")
    nc.scalar.activation(psq_scr, pooled, ACT.Square, accum_out=ns)
    py0 = small.tile([P, 1], F32, tag="py0", name="py0")
    nc.scalar.activation(py0, ns, ACT.Ln)
    nc.scalar.activation(py0, py0, ACT.Exp, scale=-0.5)
    rv = small.tile([P, 1], F32, tag="rv", name="rv")
    _emit_newton(nc, small, py0, ns, rv, "prsq", 1)
    rv_sc = small.tile([P, 1], F32, tag="rv_sc", name="rv_sc")
    nc.vector.tensor_scalar_mul(rv_sc, rv, INV_TEMP)

    # transposes feed RAW pooled sums; rv is folded into Exp's scale below
    pnT = [
        persist.tile([P, P], F32, tag=f"pnT{cb}", name=f"pnT{cb}") for cb in range(2)
    ]
    for cb in range(2):
        pq = psum_tp.tile([P, P], F32, tag="pt", name="pt")
        nc.tensor.matmul(
            pq, pooled[:, cb * P : (cb + 1) * P], identity, start=True, stop=True
        )
        nc.vector.tensor_copy(pnT[cb][:], pq)

    # ---- gram [128, 1024] + softplus/target accumulation ----
    # g holds rows of the raw-pooled gram; row b of the normalized logits
    # is rv[b] * g[b, :] (text side already normalized).
    gs = []
    es = []
    for nbk in range(2):
        g = psum_g.tile([P, 512], F32, tag="g", name="g")
        for cb in range(2):
            nc.tensor.matmul(
                g,
                pnT[cb][:],
                txtT[cb][:, nbk * 512 : (nbk + 1) * 512],
                start=(cb == 0),
                stop=(cb == 1),
            )
        e_scr = small.tile([P, 512], F32, tag=f"e_scr{nbk}", name=f"e_scr{nbk}")
        # softplus(x) = ln(exp(x) + 1); |x| <= 1/0.07 so exp can't overflow
        nc.scalar.activation(e_scr, g, ACT.Exp, scale=rv_sc)
        gs.append(g)
        es.append(e_scr)
    sp_acc = small.tile([P, 2], F32, tag="sp_acc", name="sp_acc")
    xt_acc = small.tile([P, 2], F32, tag="xt_acc", name="xt_acc")
    for nbk in range(2):
        sp_scr = small.tile([P, 512], F32, tag="sp_scr", name="sp_scr")
        nc.scalar.activation(
            sp_scr, es[nbk], ACT.Ln, bias=1.0, accum_out=sp_acc[:, nbk : nbk + 1]
        )
        xt_scr = small.tile([P, 512], F32, tag="xt_scr", name="xt_scr")
        nc.vector.tensor_mul(xt_scr, gs[nbk], tgt[:, nbk * 512 : (nbk + 1) * 512])
        nc.vector.reduce_sum(out=xt_acc[:, nbk : nbk + 1], in_=xt_scr, axis=AX.X)

    # ---- total per partition, then 128-way reduce via matmul ----
    tot = small.tile([P, 1], F32, tag="tot", name="tot")
    nc.vector.reduce_sum(out=tot, in_=sp_acc, axis=AX.X)
    xtt = small.tile([P, 1], F32, tag="xtt", name="xtt")
    nc.vector.reduce_sum(out=xtt, in_=xt_acc, axis=AX.X)
    nc.vector.tensor_mul(xtt, xtt, rv)  # fold the pooled-row norm into x*z
    nc.vector.tensor_sub(tot, tot, xtt)
    ps = psum_tp.tile([1, 1], F32, tag="ps", name="ps")
    nc.tensor.matmul(ps, tot, ones, start=True, stop=True)
    res = small.tile([1, 1], F32, tag="res", name="res")
    nc.scalar.copy(res, ps)
    nc.sync.dma_start(out=out, in_=res)


def _build_nc(reps=1, **cfg):
    nc = bacc.Bacc("TRN2", target_bir_lowering=False, debug=False, num_devices=N_CORES)
    img = nc.dram_tensor("img", [BS, C, HW], F32, kind="ExternalInput").ap()
    txt = nc.dram_tensor("txt", [B, C], F32, kind="ExternalInput").ap()
    lab_row = nc.dram_tensor("lab_row", [BS, 1], F32, kind="ExternalInput").ap()
    lab_all = nc.dram_tensor("lab_all", [1, B], F32, kind="ExternalInput").ap()
    outs = [
        nc.dram_tensor(
            "partial" if r == 0 else f"partial{r}", [1, 1], F32, kind="ExternalOutput"
        ).ap()
        for r in range(reps)
    ]

    with tile.TileContext(nc) as tc:
        with (
            tc.tile_pool(name="consts", bufs=1) as consts,
            tc.tile_pool(name="big", bufs=cfg.get("big_bufs", 6)) as big,
            tc.tile_pool(name="ascrp", bufs=2) as ascrp,
            tc.tile_pool(name="txtp", bufs=1) as txtp,
            tc.tile_pool(name="small", bufs=2) as small,
            tc.tile_pool(name="persist", bufs=1) as persist,
            tc.tile_pool(name="psum_tp", bufs=2, space="PSUM") as psum_tp,
            tc.tile_pool(name="psum_g", bufs=2, space="PSUM") as psum_g,
        ):
            identity = consts.tile([P, P], F32, tag="identity")
            make_identity(nc, identity)
            ones = consts.tile([P, 1], F32, tag="ones")
            nc.vector.memset(ones, 1.0)
            pools = (consts, big, ascrp, txtp, small, persist, psum_tp, psum_g)
            for r in range(reps):
                _emit_body(
                    nc, pools, identity, ones, img, txt, lab_row, lab_all, outs[r], cfg
                )

    nc.finalize()
    return nc


def _get_nc():
    if not _NC_CACHE:
        _NC_CACHE.append(_build_nc())
    return _NC_CACHE[0]


def make_in_maps(img_features, text_embeds, labels_f):
    img3 = img_features.reshape(B, C, HW)
    in_maps = []
    for i in range(N_CORES):
        sl = slice(i * BS, (i + 1) * BS)
        in_maps.append(
            {
                "img": img3[sl],
                "txt": text_embeds,
                "lab_row": labels_f[sl].reshape(BS, 1),
                "lab_all": labels_f.reshape(1, B),
            }
        )
    return in_maps


def kernel(img_features, text_embeds, labels):
    img_features = np.ascontiguousarray(np.asarray(img_features, dtype=np.float32))
    text_embeds = np.ascontiguousarray(np.asarray(text_embeds, dtype=np.float32))
    labels_f = np.asarray(labels).astype(np.float32)  # values < 16: exact in f32

    nc = _get_nc()
    in_maps = make_in_maps(img_features, text_embeds, labels_f)
    r = run_bass_kernel_spmd(nc, in_maps, core_ids=list(range(N_CORES)))
    total = sum(float(r.results[i]["partial"][0, 0]) for i in range(N_CORES))
    return np.float32(total / (B * B))

